# revision 26
# baseline (speedup 1.0000x reference)
"""DistSAGEConv forward on 8 Trainium2 NeuronCores (Bass/Tile).

Math (matches the reference):
    h_neigh = segment_mean(local_feats[src], dst)            # [N, D]
    out     = relu(local_feats @ W_self.T + h_neigh @ W_neigh.T + b)

Distribution: dst nodes are tiled into 391 global 128-row tiles which are
load-balanced across 8 cores x 49 slots (sorted by incident-edge count so the
SPMD per-slot chunk maximum stays near the mean); weights/bias replicated; the
feature table is replicated in every core's HBM (fp8) so remote-neighbor rows
are local indirect-DMA gathers.

Per core, per slot:
  1. dma_gather the tile's incident src rows in fp8e4 (512B/row) in edge
     order, via prepare_only descriptors + trigger_dma so the Pool engine is
     not blocked for the transfer duration.  int16 indices address the table
     as two halves split at 32768.
  2. The edge->dst one-hot selector chunks S [128e, 128dst] are precomputed
     on host as fp8 (exact 0/1) and streamed in with a plain HWDGE DMA; the
     aggregation  psum_h[dst, f] += S_pair.T @ G_pair  runs on the tensor
     engine in fp8 DoubleRow mode (two 128-edge chunks per instruction,
     0.5 cycles/row).
  3. h = psum_h * inv_deg[dst] on the scalar engine (exact fp32 scaling,
     bf16 out), PE-transpose h via bf16 identity matmuls, then
     psum_o = bias + X@Ws.T + h@Wn.T in bf16 and ReLU on the scalar engine.

All floating-point math runs on device; host preprocessing is integer edge
bookkeeping (sorting, bucketing, one-hot construction) plus dtype casts.
"""

import os

import numpy as np
import ml_dtypes

from concourse import bass, bacc, mybir, tile
from concourse.bass_utils import run_bass_kernel_spmd

F32 = mybir.dt.float32
F32R = mybir.dt.float32r
BF16 = mybir.dt.bfloat16
FP8 = mybir.dt.float8e4
I16 = mybir.dt.int16

NP_FP8 = ml_dtypes.float8_e4m3
NP_BF16 = ml_dtypes.bfloat16

N_NODES = 50000
N_EDGES = 800000
D = 512
NCORES = 8
P = 128
NTG = (N_NODES + P - 1) // P            # 391 global dst tiles
SLOTS = (NTG + NCORES - 1) // NCORES    # 49 slots per core
HALF = 32768                            # int16-addressable table boundary
GMAX = 7                                # chunks per dma_gather call (<=896 idxs)


def _cdiv(a, b):
    return (a + b - 1) // b


class Plan:
    """Compile-time structure shared by all 8 cores (program is SPMD)."""

    def __init__(self, tiles, gid, nmax):
        # tiles: per-slot (ca, cb) = 128-edge chunk counts for the low/high
        # table half, maxed across cores, padded so ca+cb is even.
        self.tiles = tiles
        self.gid = gid                   # [NCORES][SLOTS] -> global tile id
        self.nmax = nmax                 # per-slot (max_a, max_b) edge counts
        self.idx_off = []                # int16 idx column offset per slot
        self.ch_off = []                 # chunk offset per slot
        io = mo = 0
        for ca, cb in tiles:
            self.idx_off.append(io)
            self.ch_off.append(mo)
            io += (ca + cb) * 8
            mo += ca + cb
        self.sum_idx = io
        self.sum_ch = mo
        self.ch_max = max(ca + cb for ca, cb in tiles)

    def key(self):
        return tuple(self.tiles)


def _prepare(local_feats, src, dst, W_self, W_neigh, b):
    """Host-side integer preprocessing -> (plan, in_maps)."""
    feats = np.ascontiguousarray(local_feats, dtype=np.float32)
    src = np.asarray(src).astype(np.int64)
    dst = np.asarray(dst).astype(np.int64)

    deg = np.bincount(dst, minlength=N_NODES)
    inv_node = (1.0 / np.maximum(deg, 1)).astype(np.float32)

    gt = dst // P                        # global tile id per edge
    rid = (dst % P).astype(np.int16)     # row within tile
    hi = (src >= HALF).astype(np.int64)
    key = gt * 2 + hi
    order = np.argsort(key, kind="stable")
    skey = key[order]
    ssrc = src[order]
    srid = rid[order]
    bounds = np.searchsorted(skey, np.arange(NTG * 2 + 1))
    na = bounds[1::2] - bounds[:-1:2]    # per-tile low-half edge count
    nb = bounds[2::2] - bounds[1::2]

    # balance: rank the 392 slot-entries (391 real tiles + 1 dummy) by edge
    # count; slot s gets ranks [8s, 8s+8) so the per-slot max ~= mean.
    ntot = NCORES * SLOTS
    na_x = np.zeros(ntot, np.int64)
    nb_x = np.zeros(ntot, np.int64)
    na_x[:NTG] = na
    nb_x[:NTG] = nb
    rank = np.argsort(-(na_x + nb_x), kind="stable")
    gid = [[-1] * SLOTS for _ in range(NCORES)]
    tiles = []
    nmax = []
    for s in range(SLOTS):
        members = rank[8 * s:8 * s + 8]
        ma = int(max(na_x[g] for g in members))
        mb = int(max(nb_x[g] for g in members))
        ca = _cdiv(ma, P)
        cb = _cdiv(mb, P)
        if (ca + cb) % 2:
            cb += 1
        if ca + cb == 0:
            ca = cb = 1
        tiles.append((ca, cb))
        nmax.append((ma, mb))
        for c in range(NCORES):
            gid[c][s] = int(members[c])
    plan = Plan(tiles, gid, nmax)

    # replicated constants
    wts = np.ascontiguousarray(
        W_self.T.astype(np.float32).reshape(4, P, D).transpose(1, 0, 2)
    ).astype(NP_BF16)
    wtn = np.ascontiguousarray(
        W_neigh.T.astype(np.float32).reshape(4, P, D).transpose(1, 0, 2)
    ).astype(NP_BF16)
    bias = np.ascontiguousarray(b.astype(np.float32).reshape(1, D))
    ones = np.ones((1, P), dtype=np.float32)
    ident = np.eye(P, dtype=np.float32).astype(NP_BF16)
    feats8 = feats.astype(NP_FP8)

    in_maps = []
    for c in range(NCORES):
        eidx = np.zeros((P, plan.sum_idx), np.int16)
        sfp8 = np.zeros((P, plan.sum_ch, P), np.uint8)   # fp8 one-hot, via bits
        one8 = np.float32(1.0).astype(NP_FP8).view(np.uint8)
        invp = np.zeros((P, SLOTS), np.float32)
        xt = np.zeros((SLOTS, P, 4, P), NP_BF16)
        for s in range(SLOTS):
            g = gid[c][s]
            ca, cb = plan.tiles[s]
            io = plan.idx_off[s]
            mo = plan.ch_off[s]
            if g >= NTG:
                continue
            for h, base, cn in ((0, 0, ca), (1, ca, cb)):
                lo, hiq = int(bounds[2 * g + h]), int(bounds[2 * g + h + 1])
                n = hiq - lo
                nm = plan.nmax[s][h]
                npad = cn * P
                if npad == 0:
                    continue
                # pads gather row 0 (harmless); S entries there are 0
                iv = np.zeros(npad, np.int16)
                iv[:n] = (ssrc[lo:hiq] - h * HALF).astype(np.int16)
                m = iv.reshape(npad // 16, 16).T
                eidx[:, io + base * 8: io + (base + cn) * 8] = np.tile(m, (8, 1))
                if n:
                    e = np.arange(n)
                    sfp8[e % P, mo + base + e // P, srid[lo:hiq]] = one8
            r0 = g * P
            rows = min(P, N_NODES - r0)
            invp[:rows, s] = inv_node[r0:r0 + rows]
            xb = np.zeros((P, D), np.float32)
            xb[:rows] = feats[r0:r0 + rows]
            xt[s] = xb.reshape(P, 4, P).transpose(2, 1, 0).astype(NP_BF16)

        in_maps.append({
            "feats": feats8,
            "xt": xt,
            "wts": wts,
            "wtn": wtn,
            "bias": bias,
            "ones": ones,
            "ident": ident,
            "eidx": eidx,
            "sfp8": sfp8.view(NP_FP8),
            "invp": invp,
        })
    return plan, in_maps


def build(plan, mode="full"):
    """Build + compile the SPMD Bass program for one core."""
    # detect_race_conditions only affects CoreSim; the cumulative dma_sem
    # counter pattern (monotonic adds + >= waits) trips its strict checker.
    nc = bacc.Bacc("TRN2", target_bir_lowering=False, debug=False,
                   enable_asserts=False, num_devices=NCORES,
                   num_swdge_queues=4, detect_race_conditions=False)

    feats = nc.dram_tensor("feats", [N_NODES, D], FP8, kind="ExternalInput")
    xt = nc.dram_tensor("xt", [SLOTS, P, 4, P], BF16, kind="ExternalInput")
    wts = nc.dram_tensor("wts", [P, 4, D], BF16, kind="ExternalInput")
    wtn = nc.dram_tensor("wtn", [P, 4, D], BF16, kind="ExternalInput")
    bias = nc.dram_tensor("bias", [1, D], F32R, kind="ExternalInput")
    ones = nc.dram_tensor("ones", [1, P], F32R, kind="ExternalInput")
    ident = nc.dram_tensor("ident", [P, P], BF16, kind="ExternalInput")
    eidx = nc.dram_tensor("eidx", [P, plan.sum_idx], I16, kind="ExternalInput")
    sfp8 = nc.dram_tensor("sfp8", [P, plan.sum_ch, P], FP8, kind="ExternalInput")
    invp = nc.dram_tensor("invp", [P, SLOTS], F32, kind="ExternalInput")
    out = nc.dram_tensor("out", [SLOTS * P, D], F32, kind="ExternalOutput")

    AF = mybir.ActivationFunctionType
    DR = mybir.MatmulPerfMode.DoubleRow

    with tile.TileContext(nc) as tc:
        with (
            tc.tile_pool(name="const", bufs=1) as cpool,
            tc.tile_pool(name="g", bufs=3) as gpool,
            tc.tile_pool(name="s", bufs=3) as spool,
            tc.tile_pool(name="x", bufs=2) as xpool,
            tc.tile_pool(name="h", bufs=2) as hpool,
            tc.tile_pool(name="ht", bufs=2) as htpool,
            tc.tile_pool(name="o", bufs=2) as opool,
            tc.tile_pool(name="ph", bufs=2, space="PSUM") as phpool,
            tc.tile_pool(name="ptr", bufs=2, space="PSUM") as ptrpool,
            tc.tile_pool(name="po", bufs=2, space="PSUM") as popool,
        ):
            dma_sems = [nc.alloc_semaphore(f"gdma{q}") for q in range(4)]
            qctr = 0
            qcum = [0, 0, 0, 0]

            wts_s = cpool.tile([P, 4, D], BF16, tag="wts")
            nc.sync.dma_start(wts_s[:], wts[:])
            wtn_s = cpool.tile([P, 4, D], BF16, tag="wtn")
            nc.sync.dma_start(wtn_s[:], wtn[:])
            bias_s = cpool.tile([1, D], F32R, tag="bias")
            nc.sync.dma_start(bias_s[:], bias[:])
            ones_s = cpool.tile([1, P], F32R, tag="ones")
            nc.sync.dma_start(ones_s[:], ones[:])
            ident_s = cpool.tile([P, P], BF16, tag="ident")
            nc.sync.dma_start(ident_s[:], ident[:])
            idx_s = cpool.tile([P, plan.sum_idx], I16, tag="eidx")
            nc.sync.dma_start(idx_s[:], eidx[:])
            invp_s = cpool.tile([P, SLOTS], F32, tag="invp")
            nc.sync.dma_start(invp_s[:], invp[:])

            feats_a = feats[0:HALF, :]
            feats_b = feats[HALF:N_NODES, :]

            for s in range(SLOTS):
                ca, cb = plan.tiles[s]
                ch = ca + cb
                io = plan.idx_off[s]
                mo = plan.ch_off[s]

                g = gpool.tile([P, plan.ch_max, D], FP8, tag="g")
                tile_q = {}
                if mode == "nogather":
                    nc.gpsimd.memset(g[:], 0.0)
                else:
                    for hh, base, cn, src_ap in ((0, 0, ca, feats_a),
                                                 (1, ca, cb, feats_b)):
                        for c0 in range(0, cn, GMAX):
                            cw = min(GMAX, cn - c0)
                            q = qctr % 4
                            qctr += 1
                            nc.gpsimd.dma_gather(
                                g[:, base + c0:base + c0 + cw, :], src_ap,
                                idx_s[:, io + (base + c0) * 8:
                                      io + (base + c0 + cw) * 8],
                                cw * P, cw * P, D,
                                prepare_only=True, sem=dma_sems[q],
                                queue_num=q)
                            qcum[q] += 1
                            tile_q[q] = qcum[q]
                    for q in tile_q:
                        nc.gpsimd.trigger_dma(count=None, queue_num=q)

                st = spool.tile([P, plan.ch_max, P], FP8, tag="s")
                nc.sync.dma_start(st[:, 0:ch, :], sfp8[:, mo:mo + ch, :])

                xt_t = xpool.tile([P, 4, P], BF16, tag="x")
                nc.sync.dma_start(xt_t[:], xt[s])

                # aggregation: psum_h[dst, f] += S_pair.T @ G_pair (fp8 DR)
                ph = phpool.tile([P, D], F32, tag="ph")
                if mode == "noagg":
                    nc.vector.memset(ph[:], 0.0)
                else:
                    # Tile's prepare_only RAW tracking fires at desc-gen, not
                    # DMA completion -- gate the consumer on the descriptor
                    # completion sems explicitly (16 incs per gather call).
                    for q, cnt in tile_q.items():
                        nc.tensor.wait_ge(dma_sems[q], 16 * cnt)
                    npair = ch // 2
                    for pi in range(npair):
                        nc.tensor.matmul(
                            ph[:], st[:, 2 * pi:2 * pi + 2, :],
                            g[:, 2 * pi:2 * pi + 2, :],
                            start=(pi == 0), stop=(pi == npair - 1),
                            perf_mode=DR)

                # h = psum_h * inv_deg (exact fp32 scale, bf16 out)
                h = hpool.tile([P, D], BF16, tag="h")
                nc.scalar.activation(h[:], ph[:], AF.Copy,
                                     scale=invp_s[:, s:s + 1])

                # transpose h via bf16 identity matmuls
                ptr = ptrpool.tile([P, 4, P], F32, tag="ptr")
                for f in range(4):
                    nc.tensor.matmul(ptr[:, f, :], h[:, f * P:(f + 1) * P],
                                     ident_s[:], start=True, stop=True)
                ht = htpool.tile([P, 4, P], BF16, tag="ht")
                nc.vector.tensor_copy(ht[:], ptr[:])

                # out = relu(bias + X @ Ws.T + h @ Wn.T)
                po = popool.tile([P, D], F32, tag="po")
                nc.tensor.matmul(po[:], ones_s[:], bias_s[:],
                                 start=True, stop=False)
                for f in range(4):
                    nc.tensor.matmul(po[:], xt_t[:, f, :], wts_s[:, f, :],
                                     start=False, stop=False)
                    nc.tensor.matmul(po[:], ht[:, f, :], wtn_s[:, f, :],
                                     start=False, stop=(f == 3))

                o = opool.tile([P, D], F32, tag="o")
                nc.scalar.activation(o[:], po[:], AF.Relu)
                nc.sync.dma_start(out[s * P:(s + 1) * P, :], o[:])

    nc.compile()
    return nc


_cache = {}


def _get_nc(plan):
    k = plan.key()
    if k not in _cache:
        _cache[k] = build(plan)
    return _cache[k]


def _unshard(plan, results):
    out = np.empty((N_NODES, D), np.float32)
    for c in range(NCORES):
        o = results[c]["out"]
        for s in range(SLOTS):
            g = plan.gid[c][s]
            if g < 0 or g >= NTG:
                continue
            r0 = g * P
            rows = min(P, N_NODES - r0)
            out[r0:r0 + rows] = o[s * P:s * P + rows]
    return out


def kernel(local_feats, src, dst, layer=None, W_self=None, W_neigh=None,
           b=None, **_unused):
    plan, in_maps = _prepare(local_feats, src, dst, W_self, W_neigh, b)
    nc = _get_nc(plan)
    res = run_bass_kernel_spmd(nc, in_maps, core_ids=list(range(NCORES)))
    return _unshard(plan, res.results)


# revision 27
# speedup vs baseline: 1.1700x; 1.1700x over previous
"""DistSAGEConv forward on 8 Trainium2 NeuronCores (Bass/Tile).

Math (matches the reference):
    h_neigh = segment_mean(local_feats[src], dst)            # [N, D]
    out     = relu(local_feats @ W_self.T + h_neigh @ W_neigh.T + b)

Distribution: dst nodes are tiled into 391 global 128-row tiles which are
load-balanced across 8 cores x 49 slots (sorted by incident-edge count so the
SPMD per-slot chunk maximum stays near the mean); weights/bias replicated; the
feature table is replicated in every core's HBM (fp8) so remote-neighbor rows
are local indirect-DMA gathers.

Per core, per slot:
  1. dma_gather the tile's incident src rows in fp8e4 (512B/row) in edge
     order, via prepare_only descriptors + trigger_dma so the Pool engine is
     not blocked for the transfer duration.  int16 indices address the table
     as two halves split at 32768.
  2. The edge->dst one-hot selector chunks S [128e, 128dst] are precomputed
     on host as fp8 (exact 0/1) and streamed in with a plain HWDGE DMA; the
     aggregation  psum_h[dst, f] += S_pair.T @ G_pair  runs on the tensor
     engine in fp8 DoubleRow mode (two 128-edge chunks per instruction,
     0.5 cycles/row).
  3. h = psum_h * inv_deg[dst] on the scalar engine (exact fp32 scaling,
     bf16 out), PE-transpose h via bf16 identity matmuls, then
     psum_o = bias + X@Ws.T + h@Wn.T in bf16 and ReLU on the scalar engine.

All floating-point math runs on device; host preprocessing is integer edge
bookkeeping (sorting, bucketing, one-hot construction) plus dtype casts.
"""

import os

import numpy as np
import ml_dtypes

from concourse import bass, bacc, mybir, tile
from concourse.bass_utils import run_bass_kernel_spmd

F32 = mybir.dt.float32
F32R = mybir.dt.float32r
BF16 = mybir.dt.bfloat16
FP8 = mybir.dt.float8e4
I16 = mybir.dt.int16

NP_FP8 = ml_dtypes.float8_e4m3
NP_BF16 = ml_dtypes.bfloat16

N_NODES = 50000
N_EDGES = 800000
D = 512
NCORES = 8
P = 128
NTG = (N_NODES + P - 1) // P            # 391 global dst tiles
SLOTS = (NTG + NCORES - 1) // NCORES    # 49 slots per core
HALF = 32768                            # int16-addressable table boundary
GMAX = 7                                # chunks per dma_gather call (<=896 idxs)


def _cdiv(a, b):
    return (a + b - 1) // b


class Plan:
    """Compile-time structure shared by all 8 cores (program is SPMD)."""

    def __init__(self, tiles, gid, nmax):
        # tiles: per-slot (ca, cb) = 128-edge chunk counts for the low/high
        # table half, maxed across cores, padded so ca+cb is even.
        self.tiles = tiles
        self.gid = gid                   # [NCORES][SLOTS] -> global tile id
        self.nmax = nmax                 # per-slot (max_a, max_b) edge counts
        self.idx_off = []                # int16 idx column offset per slot
        self.ch_off = []                 # chunk offset per slot
        io = mo = 0
        for ca, cb in tiles:
            self.idx_off.append(io)
            self.ch_off.append(mo)
            io += (ca + cb) * 8
            mo += ca + cb
        self.sum_idx = io
        self.sum_ch = mo
        self.ch_max = max(ca + cb for ca, cb in tiles)

    def key(self):
        return tuple(self.tiles)


def _prepare(local_feats, src, dst, W_self, W_neigh, b):
    """Host-side integer preprocessing -> (plan, in_maps)."""
    feats = np.ascontiguousarray(local_feats, dtype=np.float32)
    src = np.asarray(src).astype(np.int64)
    dst = np.asarray(dst).astype(np.int64)

    deg = np.bincount(dst, minlength=N_NODES)
    inv_node = (1.0 / np.maximum(deg, 1)).astype(np.float32)

    gt = dst // P                        # global tile id per edge
    rid = (dst % P).astype(np.int16)     # row within tile
    hi = (src >= HALF).astype(np.int64)
    key = gt * 2 + hi
    order = np.argsort(key, kind="stable")
    skey = key[order]
    ssrc = src[order]
    srid = rid[order]
    bounds = np.searchsorted(skey, np.arange(NTG * 2 + 1))
    na = bounds[1::2] - bounds[:-1:2]    # per-tile low-half edge count
    nb = bounds[2::2] - bounds[1::2]

    # balance: rank the 392 slot-entries (391 real tiles + 1 dummy) by edge
    # count; slot s gets ranks [8s, 8s+8) so the per-slot max ~= mean.
    ntot = NCORES * SLOTS
    na_x = np.zeros(ntot, np.int64)
    nb_x = np.zeros(ntot, np.int64)
    na_x[:NTG] = na
    nb_x[:NTG] = nb
    rank = np.argsort(-(na_x + nb_x), kind="stable")
    gid = [[-1] * SLOTS for _ in range(NCORES)]
    tiles = []
    nmax = []
    for s in range(SLOTS):
        members = rank[8 * s:8 * s + 8]
        ma = int(max(na_x[g] for g in members))
        mb = int(max(nb_x[g] for g in members))
        ca = _cdiv(ma, P)
        cb = _cdiv(mb, P)
        if (ca + cb) % 2:
            cb += 1
        if ca + cb == 0:
            ca = cb = 1
        tiles.append((ca, cb))
        nmax.append((ma, mb))
        for c in range(NCORES):
            gid[c][s] = int(members[c])
    plan = Plan(tiles, gid, nmax)

    # replicated constants
    wts = np.ascontiguousarray(
        W_self.T.astype(np.float32).reshape(4, P, D).transpose(1, 0, 2)
    ).astype(NP_BF16)
    wtn = np.ascontiguousarray(
        W_neigh.T.astype(np.float32).reshape(4, P, D).transpose(1, 0, 2)
    ).astype(NP_BF16)
    bias = np.ascontiguousarray(b.astype(np.float32).reshape(1, D))
    ones = np.ones((1, P), dtype=np.float32)
    ident = np.eye(P, dtype=np.float32).astype(NP_BF16)
    feats8 = feats.astype(NP_FP8)

    in_maps = []
    for c in range(NCORES):
        eidx = np.zeros((P, plan.sum_idx), np.int16)
        sfp8 = np.zeros((P, plan.sum_ch, P), np.uint8)   # fp8 one-hot, via bits
        one8 = np.float32(1.0).astype(NP_FP8).view(np.uint8)
        invp = np.zeros((P, SLOTS), np.float32)
        xt = np.zeros((SLOTS, P, 4, P), NP_BF16)
        for s in range(SLOTS):
            g = gid[c][s]
            ca, cb = plan.tiles[s]
            io = plan.idx_off[s]
            mo = plan.ch_off[s]
            if g >= NTG:
                continue
            for h, base, cn in ((0, 0, ca), (1, ca, cb)):
                lo, hiq = int(bounds[2 * g + h]), int(bounds[2 * g + h + 1])
                n = hiq - lo
                nm = plan.nmax[s][h]
                npad = cn * P
                if npad == 0:
                    continue
                # pads gather row 0 (harmless); S entries there are 0
                iv = np.zeros(npad, np.int16)
                iv[:n] = (ssrc[lo:hiq] - h * HALF).astype(np.int16)
                m = iv.reshape(npad // 16, 16).T
                eidx[:, io + base * 8: io + (base + cn) * 8] = np.tile(m, (8, 1))
                if n:
                    e = np.arange(n)
                    sfp8[e % P, mo + base + e // P, srid[lo:hiq]] = one8
            r0 = g * P
            rows = min(P, N_NODES - r0)
            invp[:rows, s] = inv_node[r0:r0 + rows]
            xb = np.zeros((P, D), np.float32)
            xb[:rows] = feats[r0:r0 + rows]
            xt[s] = xb.reshape(P, 4, P).transpose(2, 1, 0).astype(NP_BF16)

        in_maps.append({
            "feats": feats8,
            "xt": xt,
            "wts": wts,
            "wtn": wtn,
            "bias": bias,
            "ones": ones,
            "ident": ident,
            "eidx": eidx,
            "sfp8": sfp8.view(NP_FP8),
            "invp": invp,
        })
    return plan, in_maps


def build(plan, mode="full"):
    """Build + compile the SPMD Bass program for one core."""
    # detect_race_conditions only affects CoreSim; the cumulative dma_sem
    # counter pattern (monotonic adds + >= waits) trips its strict checker.
    nc = bacc.Bacc("TRN2", target_bir_lowering=False, debug=False,
                   enable_asserts=False, num_devices=NCORES,
                   num_swdge_queues=4, detect_race_conditions=False)

    feats = nc.dram_tensor("feats", [N_NODES, D], FP8, kind="ExternalInput")
    xt = nc.dram_tensor("xt", [SLOTS, P, 4, P], BF16, kind="ExternalInput")
    wts = nc.dram_tensor("wts", [P, 4, D], BF16, kind="ExternalInput")
    wtn = nc.dram_tensor("wtn", [P, 4, D], BF16, kind="ExternalInput")
    bias = nc.dram_tensor("bias", [1, D], F32R, kind="ExternalInput")
    ones = nc.dram_tensor("ones", [1, P], F32R, kind="ExternalInput")
    ident = nc.dram_tensor("ident", [P, P], BF16, kind="ExternalInput")
    eidx = nc.dram_tensor("eidx", [P, plan.sum_idx], I16, kind="ExternalInput")
    sfp8 = nc.dram_tensor("sfp8", [P, plan.sum_ch, P], FP8, kind="ExternalInput")
    invp = nc.dram_tensor("invp", [P, SLOTS], F32, kind="ExternalInput")
    out = nc.dram_tensor("out", [SLOTS * P, D], F32, kind="ExternalOutput")

    AF = mybir.ActivationFunctionType
    DR = mybir.MatmulPerfMode.DoubleRow

    with tile.TileContext(nc) as tc:
        with (
            tc.tile_pool(name="const", bufs=1) as cpool,
            tc.tile_pool(name="g", bufs=4) as gpool,
            tc.tile_pool(name="s", bufs=4) as spool,
            tc.tile_pool(name="x", bufs=2) as xpool,
            tc.tile_pool(name="h", bufs=2) as hpool,
            tc.tile_pool(name="ht", bufs=2) as htpool,
            tc.tile_pool(name="o", bufs=2) as opool,
            tc.tile_pool(name="ph", bufs=2, space="PSUM") as phpool,
            tc.tile_pool(name="ptr", bufs=2, space="PSUM") as ptrpool,
            tc.tile_pool(name="po", bufs=2, space="PSUM") as popool,
        ):
            dma_sems = [nc.alloc_semaphore(f"gdma{q}") for q in range(4)]
            qctr = 0
            qcum = [0, 0, 0, 0]

            wts_s = cpool.tile([P, 4, D], BF16, tag="wts")
            nc.sync.dma_start(wts_s[:], wts[:])
            wtn_s = cpool.tile([P, 4, D], BF16, tag="wtn")
            nc.sync.dma_start(wtn_s[:], wtn[:])
            bias_s = cpool.tile([1, D], F32R, tag="bias")
            nc.sync.dma_start(bias_s[:], bias[:])
            ones_s = cpool.tile([1, P], F32R, tag="ones")
            nc.sync.dma_start(ones_s[:], ones[:])
            ident_s = cpool.tile([P, P], BF16, tag="ident")
            nc.sync.dma_start(ident_s[:], ident[:])
            idx_s = cpool.tile([P, plan.sum_idx], I16, tag="eidx")
            nc.sync.dma_start(idx_s[:], eidx[:])
            invp_s = cpool.tile([P, SLOTS], F32, tag="invp")
            nc.sync.dma_start(invp_s[:], invp[:])

            feats_a = feats[0:HALF, :]
            feats_b = feats[HALF:N_NODES, :]

            for s in range(SLOTS):
                ca, cb = plan.tiles[s]
                ch = ca + cb
                io = plan.idx_off[s]
                mo = plan.ch_off[s]

                g = gpool.tile([P, plan.ch_max, D], FP8, tag="g")
                tile_q = {}
                if mode == "nogather":
                    nc.gpsimd.memset(g[:], 0.0)
                else:
                    for hh, base, cn, src_ap in ((0, 0, ca, feats_a),
                                                 (1, ca, cb, feats_b)):
                        for c0 in range(0, cn, GMAX):
                            cw = min(GMAX, cn - c0)
                            q = qctr % 4
                            qctr += 1
                            nc.gpsimd.dma_gather(
                                g[:, base + c0:base + c0 + cw, :], src_ap,
                                idx_s[:, io + (base + c0) * 8:
                                      io + (base + c0 + cw) * 8],
                                cw * P, cw * P, D,
                                prepare_only=True, sem=dma_sems[q],
                                queue_num=q)
                            qcum[q] += 1
                            tile_q[q] = qcum[q]
                    for q in tile_q:
                        nc.gpsimd.trigger_dma(count=None, queue_num=q)

                st = spool.tile([P, plan.ch_max, P], FP8, tag="s")
                nc.sync.dma_start(st[:, 0:ch, :], sfp8[:, mo:mo + ch, :])

                xt_t = xpool.tile([P, 4, P], BF16, tag="x")
                nc.sync.dma_start(xt_t[:], xt[s])

                # aggregation: psum_h[dst, f] += S_pair.T @ G_pair (fp8 DR)
                ph = phpool.tile([P, D], F32, tag="ph")
                if mode == "noagg":
                    nc.vector.memset(ph[:], 0.0)
                else:
                    # Tile's prepare_only RAW tracking fires at desc-gen, not
                    # DMA completion -- gate the consumer on the descriptor
                    # completion sems explicitly (16 incs per gather call).
                    for q, cnt in tile_q.items():
                        nc.tensor.wait_ge(dma_sems[q], 16 * cnt)
                    npair = ch // 2
                    for pi in range(npair):
                        nc.tensor.matmul(
                            ph[:], st[:, 2 * pi:2 * pi + 2, :],
                            g[:, 2 * pi:2 * pi + 2, :],
                            start=(pi == 0), stop=(pi == npair - 1),
                            perf_mode=DR)

                # h = psum_h * inv_deg (exact fp32 scale, bf16 out)
                h = hpool.tile([P, D], BF16, tag="h")
                nc.scalar.activation(h[:], ph[:], AF.Copy,
                                     scale=invp_s[:, s:s + 1])

                # transpose h via bf16 identity matmuls
                ptr = ptrpool.tile([P, 4, P], F32, tag="ptr")
                for f in range(4):
                    nc.tensor.matmul(ptr[:, f, :], h[:, f * P:(f + 1) * P],
                                     ident_s[:], start=True, stop=True)
                ht = htpool.tile([P, 4, P], BF16, tag="ht")
                nc.vector.tensor_copy(ht[:], ptr[:])

                # out = relu(bias + X @ Ws.T + h @ Wn.T)
                po = popool.tile([P, D], F32, tag="po")
                nc.tensor.matmul(po[:], ones_s[:], bias_s[:],
                                 start=True, stop=False)
                for f in range(4):
                    nc.tensor.matmul(po[:], xt_t[:, f, :], wts_s[:, f, :],
                                     start=False, stop=False)
                    nc.tensor.matmul(po[:], ht[:, f, :], wtn_s[:, f, :],
                                     start=False, stop=(f == 3))

                o = opool.tile([P, D], F32, tag="o")
                nc.scalar.activation(o[:], po[:], AF.Relu)
                nc.sync.dma_start(out[s * P:(s + 1) * P, :], o[:])

    nc.compile()
    return nc


_cache = {}


def _get_nc(plan):
    k = plan.key()
    if k not in _cache:
        _cache[k] = build(plan)
    return _cache[k]


def _unshard(plan, results):
    out = np.empty((N_NODES, D), np.float32)
    for c in range(NCORES):
        o = results[c]["out"]
        for s in range(SLOTS):
            g = plan.gid[c][s]
            if g < 0 or g >= NTG:
                continue
            r0 = g * P
            rows = min(P, N_NODES - r0)
            out[r0:r0 + rows] = o[s * P:s * P + rows]
    return out


def kernel(local_feats, src, dst, layer=None, W_self=None, W_neigh=None,
           b=None, **_unused):
    plan, in_maps = _prepare(local_feats, src, dst, W_self, W_neigh, b)
    nc = _get_nc(plan)
    res = run_bass_kernel_spmd(nc, in_maps, core_ids=list(range(NCORES)))
    return _unshard(plan, res.results)
